# revision 121
# baseline (speedup 1.0000x reference)
"""Trainium2 Bass kernel for nn_ASCGM_30090540876360 (3x3 median-trimmed residual
between two 1x1 convs).

Math: reference computes, per (b,c,h,w), over the 9-point reflect-padded
neighborhood of d = conv1x1(x):
    diff_k = n_k - c ; absd_k = |diff_k| ; med = median9(absd)
    keep absd<=med, s = absd/max(kept absd); d3 = sum(diff*(1-s))
Since the center diff is always 0, med = 4th-smallest of the 8 neighbor
|diffs|, max(kept absd) = med, and elements with absd == med contribute 0.
Therefore exactly:
    d3 = sum_k diff_k * relu(1 - |diff_k| * rmed),  rmed = 1/max(med, ETA)
(the relu zeroes every diff with |diff| > med, and the == med case lands on
relu(0) = 0).  The ETA floor keeps the formula finite when fp16 rounding
creates >=4 zero diffs; there d3 degrades gracefully to 0, matching the
reference at such near-degenerate pixels.

Sharding: data-parallel over batch B=8 across the 8 NeuronCores (1 image per
core).  On-core layout: 128 partitions = 2 image halves x 64 channels; each
partition holds 64 rows (+1 halo row each side) of one half.  Both halves are
processed by single instructions via block-diagonal conv weights.

Stencil runs in fp16 (DVE 2x mode) against a dual-copy padded d buffer
(dpadE / dpadO shifted by one element) so every strided fp16 operand stays
4-byte aligned.

Engine split (DVE is the bottleneck at ~92%; everything else is offloaded):
  PE : conv1; three of the four diff pairs as +/-identity matmuls over the
       padded d buffers (chunks >= 1); conv2 accumulates
       w2a^T d + sum_k w2b^T g_k in one PSUM pass, so the 8-way k-sum of
       the trimmed residual is free PE work.
  ACT: PSUM evacuations with bias; raw-diff copies for PE-made planes
       (k-major, so each chunk-granular abs unblocks after one plane's 4
       copies); all |diff| planes from SBUF.
  DVE: diff pair 0 (fused 2-wide subtract), 24-op median-selection
       network with interleaved sort3 chains, ANT_MAXMAX_RCP (final
       comparator + ETA floor + 1-Newton reciprocal fused in one 7-node
       op), and ANT_TRIM_G (g = d*relu(1 - |d|*rmed)) in place over the
       8 diff planes.

Software pipeline per chunk n (double-buffered dstack):
  DVE: network(n) -> maxmax+rcp(n) -> diffs(n+1) -> g-groups(n)
  PE : conv1 pieces(n+2), PE-diffs(n+1), conv2(n) per 4-row group
  ACT: evacuations(n+2), diff/abs planes(n+1) interleaved with group
       evacuations(n); per-group merged-half DMA-out rides one 4-D AP.
"""
import sys, os
sys.path.insert(0, '/opt/trn_rl_repo')

import numpy as np
from contextlib import ExitStack

import concourse.bass as bass
import concourse.tile as tile
from concourse import bacc, mybir
from concourse.bass_utils import run_bass_kernel_spmd
from concourse import dve_ops as _dve_ops
from concourse.dve_spec import (Spec, Src0, Src1, C0, Zero, One, maxx, minn,
                                lower, scan, AluOp)
from concourse.dve_spec import _has_src1 as has_src1
from concourse.dve_uop import DveOpSpec


def _register_dve_op(name, spec):
    for op in _dve_ops.OPS:
        if op.name == name:
            return op
    shas = {}
    op = _dve_ops.DveOp(name, spec, subdim=False, uops_sha=shas)
    _dve_ops.OPS.append(op)
    _dve_ops._SUB_OPCODE_FOR_NAME[name] = (_dve_ops._CUSTOM_DVE_ROW_BASE
                                           + len(_dve_ops.OPS) - 1)
    _dve_ops.CUSTOM_DVE_SPECS[name] = spec
    for ver in ("v3", "v4"):
        r = DveOpSpec(name=name, opcode=_dve_ops.get_dve_sub_opcode(name),
                      uops=lower(spec, ver=ver), rd1_en=has_src1(spec))
        shas[ver] = r.sha(ver)
    return op


def _register_trim_g():
    """Custom DVE op: out = in0 * relu(in1 - |in0|) * approx(1/in1).

    in1 = med+floor (fp32, broadcast along k); the reciprocal is inlined
    as the fp32 BITWISE_NOT exponent-flip seed plus one Newton step
    (~1.2e-3 rel err, far below the fp16 noise floor), so the trimmed
    weight g_k = diff_k * (1 - s_k) costs no separate reciprocal pass.
    The k-sum then rides the conv2 PSUM accumulation on PE."""
    import numpy as _np
    t = Src0 * maxx(One - maxx(Src0, Zero - Src0) * Src1, Zero)

    def _ref(in0, in1, *a):
        sh = in0.shape
        x = _np.asarray(in0, _np.float64).reshape(sh[0], -1)
        m = _np.asarray(in1, _np.float64)
        m = (m.reshape(x.shape) if m.size == x.size
             else _np.broadcast_to(m.reshape(m.shape[0], -1), x.shape))
        return (x * _np.maximum(1.0 - _np.abs(x) * m, 0.0)).reshape(sh)

    return _register_dve_op("ANT_TRIM_G", Spec(body=t, reference=_ref))


TRIMG = _register_trim_g()
RCP_C0 = -0.23549792
RCP_C1 = 2.0017324


def _register_maxmax_rcp():
    """out = approx(1 / max(max(in0, in1), s0)) — the median network's final
    comparator, the ETA floor, and the reciprocal (fp32 exponent-flip seed +
    one Newton step, ~1.2e-3 rel err) fused into a single 7-node DVE op.
    Replaces a separate comparator + reciprocal pass."""
    import numpy as _np
    from concourse.dve_spec import Bin, C1, C2
    m = maxx(maxx(Src0, Src1), C0)
    r0 = Bin(AluOp.BITWISE_NOT, m, m) * C1
    r1 = r0 * (C2 - m * r0)

    def _ref(in0, in1, c0, c1, imm2, *a):
        mm = _np.maximum(_np.maximum(in0, in1.reshape(in0.shape)), c0)
        return 1.0 / mm

    return _register_dve_op(
        "ANT_MAXMAX_RCP", Spec(body=r1, reference=_ref))


MAXMAXRCP = _register_maxmax_rcp()


def _register_abs():
    """out = |in0| — used on the DVE only for chunk 0's last-consumed
    planes, where the ACT's serial abs chain gates the first network and
    the DVE is otherwise idle in the prologue fill."""
    import numpy as _np
    return _register_dve_op(
        "ANT_ABS", Spec(body=maxx(Src0, Zero - Src0),
                        reference=lambda in0, *a: _np.abs(in0)))


ABSOP = _register_abs()


def _register_maxmax_eta():
    """out = max(max(in0, in1), s0) — the median network's final comparator
    fused with the ETA floor, emitting fp32 for the reciprocal directly
    (skips two ACT cast hops that stall the DVE pipeline)."""
    import numpy as _np

    def _ref(in0, in1, c0, *a):
        return _np.maximum(_np.maximum(in0, in1.reshape(in0.shape)), c0)

    return _register_dve_op(
        "ANT_MAXMAX_ETA", Spec(body=maxx(maxx(Src0, Src1), C0), reference=_ref))


MAXMAXETA = _register_maxmax_eta()

F16 = mybir.dt.float16
F32 = mybir.dt.float32
ALU = mybir.AluOpType
AFT = mybir.ActivationFunctionType

C = 64          # channels
H = W = 128     # image size
NCORES = 8
PR = 66         # padded rows per half (64 + halo)
WP = 130        # padded row width
ETA = 1e-4      # median floor (fp16-safe; see module docstring)
RCH = 16        # stencil chunk rows (per half) -> 4 chunks

# schedule variants (swept via TimelineSim; defaults = best found)
V_MMETA = int(os.environ.get("V_MMETA", "1"))    # 1: fused custom maxmax+eta
V_DIFFS_EARLY = int(os.environ.get("V_DIFFS_EARLY", "1"))  # diffs(n+1) pre-recip
V_ABS_IL = int(os.environ.get("V_ABS_IL", "1"))  # abs(n+1) interleaved w/ evacs
V_PEDIFF = int(os.environ.get("V_PEDIFF", "3"))  # diff pairs via PE+ACT (0-4)
V_PEDIFF_FROM = int(os.environ.get("V_PEDIFF_FROM", "1"))  # first chunk w/ PE
V_ABS_SBUF = int(os.environ.get("V_ABS_SBUF", "1"))  # PE-plane abs from SBUF
V_ABS_IN_DIFFS = int(os.environ.get("V_ABS_IN_DIFFS", "0"))  # abs rides diffs
V_DVE_ABS0 = int(os.environ.get("V_DVE_ABS0", "0"))  # chunk-0 abs 4,7 on DVE
V_GSEG = int(os.environ.get("V_GSEG", "0"))  # pair conv2 groups per g op
V_O_PSUM = int(os.environ.get("V_O_PSUM", "0"))  # prologue dpadO from PSUM
# (V_O_PSUM=1 measured +1.1us: the 1x PSUM-read tensor_scalar and longer
# PSUM occupancy outweigh starting one sem-hop earlier)
V_CH0 = int(os.environ.get("V_CH0", "8"))  # first-chunk rows (4 or 8)
V_EODVE = int(os.environ.get("V_EODVE", "1"))  # chunk-1 dpadO on DVE too
V_CPD = int(os.environ.get("V_CPD", "1"))  # prologue colpads on DVE
V_O_BULK = int(os.environ.get("V_O_BULK", "0"))  # one bulk dpadO copy
# (V_O_BULK=1 measured +2.8us: per-piece copies let the DVE wait-queue start
# early diff pairs as their rows land; one bulk copy serializes that)
# (V_GSEG=1 measured +12us: the per-group g->conv2 pipelining outweighs
# the saved instruction overhead; kept only as a swept-and-rejected variant)


def pe_ks(cidx):
    # planes the PE produces for this chunk (pairs taken from the END of the
    # DVE pair list); early chunks stay all-DVE -- the DVE is idle during the
    # prologue fill, so offloading there only adds ACT latency
    n = V_PEDIFF if cidx >= V_PEDIFF_FROM else 0
    return [k for pi in range(4 - n, 4) for k in (2 * pi, 2 * pi + 1)]
# plane k -> (dpad buffer, row offset, first column of the 128-col window)
PEPLANES = {0: ('E', -1, 0), 1: ('E', -1, 2), 2: ('E', 1, 0),
            3: ('E', 1, 2), 4: ('E', 0, 0), 5: ('E', 0, 2),
            6: ('O', -1, 2), 7: ('O', 1, 2)}


def build_program():
    nc = bacc.Bacc("TRN2", target_bir_lowering=False, debug=False)

    x16 = nc.dram_tensor("x16", [C, H, W], F16, kind="ExternalInput")
    w1bd = nc.dram_tensor("w1bd", [128, 128], F16, kind="ExternalInput")
    w2abd = nc.dram_tensor("w2abd", [128, 128], F16, kind="ExternalInput")
    w2bbd = nc.dram_tensor("w2bbd", [128, 128], F16, kind="ExternalInput")
    identd = nc.dram_tensor("identd", [128, 128], F16, kind="ExternalInput")
    identnd = nc.dram_tensor("identnd", [128, 128], F16, kind="ExternalInput")
    b1v = nc.dram_tensor("b1v", [128, 1], F32, kind="ExternalInput")
    b2v = nc.dram_tensor("b2v", [128, 1], F32, kind="ExternalInput")
    out = nc.dram_tensor("out", [C, H, W], F32, kind="ExternalOutput")

    v = nc.vector
    s = nc.scalar

    with tile.TileContext(nc) as tc:
        with ExitStack() as ctx:
            cpool = ctx.enter_context(tc.tile_pool(name="const", bufs=1))
            w1sb = cpool.tile([128, 128], F16, tag="w1sb")
            w2asb = cpool.tile([128, 128], F16, tag="w2asb")
            w2bsb = cpool.tile([128, 128], F16, tag="w2bsb")
            identsb = cpool.tile([128, 128], F16, tag="identsb")
            identnsb = cpool.tile([128, 128], F16, tag="identnsb")
            b1sb = cpool.tile([128, 1], F32, tag="b1sb")
            b2sb = cpool.tile([128, 1], F32, tag="b2sb")
            etasb = cpool.tile([128, 1], F32, tag="etasb")
            # Trigger the ACT function-table load (~1.3us) at t=0 on a dummy
            # op; otherwise it lazily precedes the first conv evacuation and
            # inherits its sem waits, stretching the prologue.
            warm = cpool.tile([128, 1], F32, tag="warm")
            s.memzero(warm[:])
            s.activation(warm[:], warm[:], AFT.Abs)

            dpool = ctx.enter_context(tc.tile_pool(name="dpad", bufs=1))
            dpadE = dpool.tile([128, PR, WP], F16, tag="dpadE")
            dpadO = dpool.tile([128, PR, WP], F16, tag="dpadO")

            # ---- load x with halo rows (reflection handled by duplicate DMAs)
            xpool = ctx.enter_context(tc.tile_pool(name="xp", bufs=1))
            xsb = xpool.tile([128, PR, W], F16, tag="xsb")
            # half A: global rows -1..64 -> local 0..65 (row -1 == row 1)
            # half B: global rows 63..128 -> local 0..65 (row 128 == row 126)
            # First pieces small (conv chunk 0 needs only local rows 0..5).
            # w1/b1 ride the SWDGE generator (parallel to HWDGE); the HWDGE
            # queue leads with the x rows the first conv pieces need.
            nc.gpsimd.dma_start(w1sb[:], w1bd[:])
            nc.gpsimd.dma_start(b1sb[:], b1v[:])
            nc.gpsimd.memset(etasb[:], ETA)
            nc.sync.dma_start(xsb[0:64, 0:1, :], x16[:, 1:2, :])
            nc.sync.dma_start(xsb[0:64, 1:8, :], x16[:, 0:7, :])
            nc.sync.dma_start(xsb[64:128, 0:8, :], x16[:, 63:71, :])
            nc.sync.dma_start(xsb[0:64, 8:20, :], x16[:, 7:19, :])
            nc.sync.dma_start(xsb[64:128, 8:20, :], x16[:, 71:83, :])
            nc.gpsimd.dma_start(w2asb[:], w2abd[:])
            nc.gpsimd.dma_start(w2bsb[:], w2bbd[:])
            nc.gpsimd.dma_start(b2sb[:], b2v[:])
            nc.gpsimd.dma_start(identsb[:], identd[:])
            nc.gpsimd.dma_start(identnsb[:], identnd[:])
            nc.sync.dma_start(xsb[64:128, 65:66, :], x16[:, 126:127, :])
            for rr in range(19, 65, 16):  # bulk loads, alternating queues
                ra = min(rr + 16, 65)   # half A: local 1+rr <- global rr
                rb = min(rr + 16, 64)   # half B: local 1+rr <- global 64+rr
                nc.gpsimd.dma_start(xsb[0:64, 1 + rr:1 + ra, :],
                                    x16[:, rr:ra, :])
                if rb > rr:
                    nc.sync.dma_start(xsb[64:128, 1 + rr:1 + rb, :],
                                      x16[:, 64 + rr:64 + rb, :])

            # ---- conv1 producers (emitted per consumer chunk, see loop)
            pp1 = ctx.enter_context(tc.tile_pool(name="psum1", bufs=2,
                                                 space="PSUM"))
            ppd = ctx.enter_context(tc.tile_pool(
                name="psumd", bufs=int(os.environ.get("V_PPD", "2")),
                space="PSUM"))
            xflat = xsb[:].rearrange("p r w -> p (r w)")

            def conv1_chunk(r0, nr, e_on_dve=False):
                # conv1 of padded rows r0..r0+nr
                n0 = W * r0
                nsz = W * nr
                ps = pp1.tile([128, nr, W], F32, name="ps1", tag="ps1")
                nc.tensor.matmul(ps[:], w1sb[:], xflat[:, n0:n0 + nsz],
                                 start=True, stop=True)
                # evacuate with bias, fp32->fp16, into both shifted pads,
                # including the reflected column pads straight from PSUM
                s.add(dpadE[:, r0:r0 + nr, 1:129], ps[:], b1sb[:])
                if e_on_dve:
                    # prologue only: DVE is idle, so build the odd-shifted
                    # copy there; sourcing straight from PSUM (bias via
                    # tensor_scalar) starts one sem-hop earlier than copying
                    # the finished dpadE rows and runs in parallel with the
                    # E-evacuation. Column pads for these rows are emitted
                    # afterwards from dpadE so they neither occupy ACT's
                    # queue between the gating E-evacuations nor hold PSUM.
                    if V_O_PSUM:
                        v.tensor_scalar(dpadO[:, r0:r0 + nr, 2:130], ps[:],
                                        b1sb[:], None, ALU.add)
                    elif not V_O_BULK:
                        v.tensor_copy(dpadO[:, r0:r0 + nr, 2:130],
                                      dpadE[:, r0:r0 + nr, 1:129])
                else:
                    s.add(dpadO[:, r0:r0 + nr, 2:130], ps[:], b1sb[:])
                    s.add(dpadE[:, r0:r0 + nr, 0:130:129],
                          ps[:, :, 1:127:125], b1sb[:])

            # stencil chunks (interior row ranges): two 8-row chunks first so
            # the whole subs/abs/network pipeline warms up at half
            # granularity, then 16-row steady state.  conv1 pieces cover each
            # chunk's padded rows (plus halo) right before it, with 2-row
            # pieces in the prologue ladder.
            if V_CH0 == 4:
                CHUNKS = [(0, 4), (4, 12), (16, 16), (32, 16), (48, 16)]
                CONV_PIECES = {
                    0: [(0, 2), (2, 2), (4, 2)],
                    4: [(6, 4), (10, 4), (14, 4)],
                    16: [(18, 4), (22, 4), (26, 4), (30, 4)],
                    32: [(34, 4), (38, 4), (42, 4), (46, 4)],
                    48: [(50, 4), (54, 4), (58, 4), (62, 4)],
                }
                NCOLPAD = 6
            else:
                CHUNKS = [(0, 8), (8, 8), (16, 16), (32, 16), (48, 16)]
                CONV_PIECES = {
                    0: [(0, 2), (2, 2), (4, 2), (6, 2), (8, 2)],
                    8: [(10, 4), (14, 4)],
                    16: [(18, 4), (22, 4), (26, 4), (30, 4)],
                    32: [(34, 4), (38, 4), (42, 4), (46, 4)],
                    48: [(50, 4), (54, 4), (58, 4), (62, 4)],
                }
                NCOLPAD = 10

            # ---- stencil + conv2, chunked over rows
            spool = ctx.enter_context(tc.tile_pool(name="sten", bufs=1))
            # dstack is double-buffered: chunk n+1's diffs would otherwise
            # wait for chunk n's conv2 matmuls (PE) to finish reading g.
            dkpool = ctx.enter_context(tc.tile_pool(name="dk", bufs=2))
            opool = ctx.enter_context(tc.tile_pool(name="outp", bufs=1))
            pp2 = ctx.enter_context(tc.tile_pool(
                name="psum2", bufs=int(os.environ.get("V_PP2", "4")),
                space="PSUM"))

            def produce(cidx):
                # conv pieces for chunk cidx (they gate its stencil diffs)
                pcr0 = CHUNKS[cidx][0]
                for pr0, pnr in CONV_PIECES[pcr0]:
                    conv1_chunk(pr0, pnr,
                                e_on_dve=(cidx <= (1 if V_EODVE else 0)))
                cpad = v.tensor_copy if V_CPD else s.copy
                if V_O_BULK and cidx <= (1 if V_EODVE else 0):
                    # one 4x-rate bulk dpadO copy after the evacuation
                    # ladder instead of five gap-ridden per-piece copies
                    rows = slice(0, NCOLPAD) if cidx == 0 else slice(10, 18)
                    v.tensor_copy(dpadO[:, rows, 2:130],
                                  dpadE[:, rows, 1:129])
                if cidx == 1 and V_EODVE and V_CH0 == 8:
                    # chunk 1's pieces also ran the DVE-copy path during the
                    # ACT-gated prologue; emit its deferred column pads
                    cpad(dpadE[:, 10:18, 0:130:129],
                         dpadE[:, 10:18, 2:128:125])
                if cidx == 0:
                    # deferred prologue column pads, sourced from dpadE
                    cpad(dpadE[:, 0:NCOLPAD, 0:130:129],
                         dpadE[:, 0:NCOLPAD, 2:128:125])

            def newt(tag, dt=F16):
                return spool.tile([128, RCH, W], dt, name=tag, tag=tag)

            absd = [newt(f"absd{k}") for k in range(8)]
            p0, q0 = newt("p0"), newt("q0")
            r0_, r1_ = newt("r0_"), newt("r1_")
            medf = newt("medf", F32)
            dstacks = {}

            def emit_diffs(cidx):
                # diffs (DVE) for chunk cidx
                cr0, cnr = CHUNKS[cidx]
                rr0 = 1 + cr0
                dstack = dkpool.tile([128, 8, RCH, W], F16, name="dstack",
                                     tag="dstack")
                dstacks[cidx] = dstack
                # 4 fused subtracts, 2 neighbors each (k-dim in the AP); the
                # center operand broadcasts over k with a 0-stride dim.
                eflat = dpadE[:].rearrange("p r w -> p (r w)")
                oflat = dpadO[:].rearrange("p r w -> p (r w)")
                pairs = [  # (src flat view, base row offset, k step)
                    (eflat, rr0 - 1, 0, 2),        # (0,0),(0,2)
                    (eflat, rr0 + 1, 0, 2),        # (2,0),(2,2)
                    (eflat, rr0, 0, 2),            # (1,0),(1,2)
                    (oflat, rr0 - 1, 2, 2 * WP),   # (0,1),(2,1)
                ]
                kset = pe_ks(cidx)
                pairs = pairs[:4 - len(kset) // 2]
                ctrb = bass.AP(tensor=oflat.tensor, offset=rr0 * WP + 2,
                               ap=[oflat.ap[0], [0, 2], [WP, cnr], [1, W]])
                for pi, (src, brow, bcol, kst) in enumerate(pairs):
                    nb2 = bass.AP(tensor=src.tensor,
                                  offset=brow * WP + bcol,
                                  ap=[src.ap[0], [kst, 2], [WP, cnr], [1, W]])
                    v.tensor_tensor(dstack[:, 2 * pi:2 * pi + 2, 0:cnr],
                                    nb2, ctrb, ALU.subtract)
                # remaining planes on PE via +/- identity matmuls over the
                # padded d buffers; ACT evacuates the raw diff (for g) from
                # PSUM, freeing DVE subtracts.  k-major order: the
                # chunk-granular abs of plane k needs ALL its groups' copies,
                # so finishing one plane at a time unblocks each abs after 4
                # copies instead of after nearly all 24.
                for k in kset:
                    srcn, roff, clo = PEPLANES[k]
                    srcb = dpadE if srcn == 'E' else dpadO
                    for gr in range(0, cnr, 4):
                        rg = rr0 + gr
                        psd = ppd.tile([128, 4, W], F32, name="psd",
                                       tag="psd")
                        nc.tensor.matmul(
                            psd[:], identsb[:],
                            srcb[:, rg + roff:rg + roff + 4, clo:clo + 128],
                            start=True, stop=False)
                        nc.tensor.matmul(
                            psd[:], identnsb[:],
                            dpadO[:, rg:rg + 4, 2:130],
                            start=False, stop=True)
                        s.copy(dstack[:, k, gr:gr + 4, :], psd[:])
                        if not V_ABS_SBUF:
                            s.activation(absd[k][:, gr:gr + 4], psd[:],
                                         AFT.Abs)
                    if V_ABS_IN_DIFFS:
                        s.activation(absd[k][:, 0:cnr], dstack[:, k, 0:cnr],
                                     AFT.Abs)

            def emit_abs(cidx, k0=0, k1=8):
                # |diffs| planes [k0, k1) (ACT) for chunk cidx -- interleaved
                # between the previous chunk's group evacuations so neither
                # the abs stream nor PSUM recycling parks the other in ACT's
                # 4-deep wait queue
                cr0, cnr = CHUNKS[cidx]
                dstack = dstacks[cidx]
                # chunk 0: ACT's serial abs chain gates the first network;
                # order by network consumption and let the idle DVE take the
                # two last-consumed planes itself
                order = (0, 1, 2, 3, 5, 6, 4, 7) if cidx == 0 else \
                    tuple(range(8))
                for k in order[k0:k1]:
                    if k in pe_ks(cidx) and (V_ABS_IN_DIFFS
                                             or not V_ABS_SBUF):
                        continue   # PE planes: abs already emitted
                    if cidx == 0 and k in (4, 7) and V_DVE_ABS0:
                        v._custom_dve(ABSOP, out=absd[k][:, 0:cnr],
                                      in0=dstack[:, k, 0:cnr])
                        continue
                    s.activation(absd[k][:, 0:cnr], dstack[:, k, 0:cnr],
                                 AFT.Abs)

            # Software pipeline: per iteration the DVE runs
            #   network(n) -> ttmax(n) -> diffs(n+1) -> recip(n) -> g(n)
            # so the ACT hop (medf = med+ETA) hides behind diffs(n+1), and
            # abs(n+1) lands while the DVE is busy with recip+g of chunk n.
            # Conv pieces are produced two chunks ahead to cover diffs(n+1).
            produce(0)
            emit_diffs(0)
            emit_abs(0)
            produce(1)
            for cidx, (cr0, cnr) in enumerate(CHUNKS):
                first = cidx == 0
                last = cidx == len(CHUNKS) - 1
                rr0 = 1 + cr0               # first interior padded row of chunk
                dstack = dstacks.pop(cidx)

                def network(eng, r0r, r1r):
                    def tt(op, a, b, o):
                        eng.tensor_tensor(o[:, r0r:r1r], a[:, r0r:r1r],
                                          b[:, r0r:r1r], op)
                        return o

                    # 24-op selection of 4th-smallest-of-8 via median-of-9
                    # (the 9th value is the always-zero center diff):
                    # med9 = med3( max3(mins), med3(mids), min3(maxs) ) over
                    # three sorted triples T0=(0,a0,a1), T1=(a2,a3,a4),
                    # T2=(a5,a6,a7).  The two sort3 chains are interleaved so
                    # dependent ops sit >=2 apart and the write-ack latency
                    # is hidden.  absd slots are reused as scratch once dead.
                    A = absd
                    tt(ALU.min, A[0], A[1], p0)
                    tt(ALU.min, A[2], A[3], r0_)
                    tt(ALU.min, A[5], A[6], r1_)
                    tt(ALU.max, A[0], A[1], q0)
                    tt(ALU.max, A[2], A[3], A[0])
                    tt(ALU.max, A[5], A[6], A[1])
                    tt(ALU.max, A[0], A[4], A[2])   # hi1
                    tt(ALU.max, A[1], A[7], A[5])   # hi2
                    tt(ALU.min, A[0], A[4], A[3])
                    tt(ALU.min, A[1], A[7], A[6])
                    tt(ALU.min, r0_, A[3], A[4])    # lo1
                    tt(ALU.min, r1_, A[6], A[7])    # lo2
                    tt(ALU.max, r0_, A[3], A[0])    # mi1
                    tt(ALU.max, r1_, A[6], A[1])    # mi2
                    tt(ALU.max, A[4], A[7], A[3])   # mxlo = max(lo1, lo2)
                    tt(ALU.min, A[2], A[5], A[4])   # min(hi1, hi2)
                    tt(ALU.min, A[0], A[1], A[6])   # m_ab = min(mi1, mi2)
                    tt(ALU.max, A[0], A[1], A[2])   # M_ab = max(mi1, mi2)
                    tt(ALU.min, A[2], p0, A[0])     # t = min(M_ab, p0)
                    tt(ALU.min, A[4], q0, A[5])     # mnhi = min(., q0)
                    tt(ALU.max, A[6], A[0], A[1])   # v = med3(mids)
                    tt(ALU.min, A[3], A[1], A[4])   # f_ab = min(mxlo, v)
                    tt(ALU.max, A[3], A[1], A[7])   # F_ab = max(mxlo, v)
                    tt(ALU.min, A[7], A[5], A[6])   # t2 = min(F_ab, mnhi)
                    return A[4], A[6]               # f_ab, t2

                fabD, t2D = network(v, 0, cnr)
                if V_MMETA:
                    # final comparator + ETA floor + reciprocal fused in one
                    # DVE op: medf = approx 1/max(max(f_ab, t2), ETA).
                    # Flat 1-D views: the TTSS struct (imm2) needs 1D src1.
                    flat = lambda tl: tl[:].rearrange(
                        "p r w -> p (r w)")[:, 0:cnr * W]
                    v._custom_dve(MAXMAXRCP, out=flat(medf),
                                  in0=flat(fabD), in1=flat(t2D),
                                  s0=ETA, s1=RCP_C0, imm2=RCP_C1)
                else:
                    # final comparator at fp16 2x; ETA floor rides the
                    # fp16->fp32 cast on ACT as med+ETA (within error budget)
                    med16 = absd[0]
                    v.tensor_tensor(med16[:, 0:cnr], fabD[:, 0:cnr],
                                    t2D[:, 0:cnr], ALU.max)
                    s.add(medf[:, 0:cnr], med16[:, 0:cnr], etasb[:])
                if cidx + 2 < len(CHUNKS):
                    produce(cidx + 2)
                if V_DIFFS_EARLY and not last:
                    emit_diffs(cidx + 1)

                # ---- g = d*relu(1 - |d|*rmed) in place on the diff planes,
                # then conv2: out = w2a^T d + sum_k w2b^T g_k + b2 rides one
                # PSUM accumulation per group.  Per-4-row groups so the tail
                # pipeline starts immediately; the last chunk ends with 2-row
                # groups to shorten the exit chain.
                dsf = dstack[:].rearrange("p k r w -> p (k r w)")
                rmf = medf[:].rearrange("p r w -> p (r w)")
                osb = opool.tile([128, RCH, W], F32, name="osb", tag="osb")
                TAILS = {0: [(0, 4), (4, 4), (8, 4), (12, 3), (15, 1)],
                         1: [(0, 4), (4, 4), (8, 4), (12, 2), (14, 2)],
                         2: [(0, 4), (4, 4), (8, 4), (12, 4)],
                         3: [(0, 4), (4, 4), (8, 4), (12, 2), (14, 1),
                             (15, 1)]}
                groups = [(g, 4) for g in range(0, cnr, 4)] if not last else \
                    TAILS[int(os.environ.get("V_TAIL", "3"))]
                # g-op emission is coarser than the conv2 groups (the
                # matmuls read 4-row slices of a wider g output): pairing
                # groups halves the g instruction count; the last chunk's
                # final two groups stay fine-grained for a short exit chain.
                gsegs = {}
                merge = groups if not last or not V_GSEG else groups[:-2]
                i = 0
                while i < len(merge):
                    if V_GSEG and i + 1 < len(merge):
                        a, b = merge[i], merge[i + 1]
                        gsegs[a[0]] = a[1] + b[1]
                        i += 2
                    else:
                        gsegs[merge[i][0]] = merge[i][1]
                        i += 1
                if last and V_GSEG:
                    for gr0, gnr in groups[-2:]:
                        gsegs[gr0] = gnr
                for gi, (gr0, gnr) in enumerate(groups):
                    if gr0 in gsegs:
                        off = W * gr0
                        npx = W * gsegs[gr0]
                        gin = bass.AP(tensor=dsf.tensor,
                                      offset=dsf.offset + off,
                                      ap=[dsf.ap[0], [RCH * W, 8], [1, npx]])
                        rin = bass.AP(tensor=rmf.tensor,
                                      offset=rmf.offset + off,
                                      ap=[rmf.ap[0], [0, 8], [1, npx]])
                        v._custom_dve(TRIMG, out=gin, in0=gin, in1=rin,
                                      s0=RCP_C0, s1=RCP_C1)
                    sl = slice(gr0, gr0 + gnr)
                    r0 = rr0 + gr0
                    ps2 = pp2.tile([128, gnr, W], F32, name="ps2", tag="ps2")
                    nc.tensor.matmul(ps2[:], w2asb[:],
                                     dpadE[:, r0:r0 + gnr, 1:129],
                                     start=True, stop=False)
                    for k in range(8):
                        nc.tensor.matmul(ps2[:], w2bsb[:],
                                         dstack[:, k, gr0:gr0 + gnr, :],
                                         start=False, stop=(k == 7))
                    s.add(osb[:, sl], ps2[:], b2sb[:])
                    # per-group output DMA (frees osb rows for the next
                    # chunk's evacuations immediately); both halves ride one
                    # DMA via a 4-D DRAM AP [half, c, row, w]
                    g0 = cr0 + gr0
                    ov = out[:]
                    od = bass.AP(tensor=ov.tensor, offset=g0 * W,
                                 ap=[[64 * W, 2], [H * W, 64], [W, gnr],
                                     [1, W]])
                    nc.sync.dma_start(od, osb[:, sl])
                    if V_ABS_IL and not last:
                        ng = len(groups)
                        emit_abs(cidx + 1, gi * 8 // ng,
                                 (gi + 1) * 8 // ng)

                if not last:
                    if not V_DIFFS_EARLY:
                        emit_diffs(cidx + 1)
                    if not V_ABS_IL:
                        emit_abs(cidx + 1)

    nc.compile()
    return nc


_NC_CACHE = None


def _get_program():
    global _NC_CACHE
    if _NC_CACHE is None:
        _NC_CACHE = build_program()
    return _NC_CACHE


def _host_inputs(x, w1, b1, w2, b2):
    """Build the per-core input maps (shard by batch, prep weights)."""
    f16 = np.float16
    w1t = np.ascontiguousarray(w1.T)                        # (c_in, c_out)
    w2at = np.ascontiguousarray(w2[:, :C].T)                # (c, o)
    w2bt = np.ascontiguousarray(w2[:, C:].T)
    bd = lambda m: np.block([[m, np.zeros_like(m)], [np.zeros_like(m), m]]).astype(f16)
    w1bd, w2abd, w2bbd = bd(w1t), bd(w2at), bd(w2bt)
    identbd = np.eye(128, dtype=f16)
    b1v = np.concatenate([b1, b1]).astype(np.float32).reshape(128, 1)
    b2v = np.concatenate([b2, b2]).astype(np.float32).reshape(128, 1)
    in_maps = []
    for i in range(NCORES):
        in_maps.append({
            "x16": np.ascontiguousarray(x[i].astype(f16)),
            "w1bd": w1bd, "w2abd": w2abd, "w2bbd": w2bbd,
            "identd": identbd, "identnd": -identbd,
            "b1v": b1v, "b2v": b2v,
        })
    return in_maps


def _spot_check(out, x, w1, b1, w2, b2, b=0, h=5):
    """Host-side reference for one output row; guards against the axon
    relay's rare whole-invocation garbage (seen once: rel ~37 vs 6e-3,
    clean on rerun).  Loose threshold: true fp16 rel_l2 is ~4e-3."""
    d = np.einsum('oc,chw->ohw', w1, x[b, :, h - 1:h + 2, :]) \
        + b1[:, None, None]
    p = np.pad(d, ((0, 0), (0, 0), (1, 1)), mode='reflect')
    nb = np.stack([p[:, i, j:j + 128] for i in range(3) for j in range(3)],
                  axis=-1)
    diff = nb - d[:, 1:2, :].transpose(0, 2, 1)
    absd = np.abs(diff)
    med = np.median(absd, axis=-1, keepdims=True)
    keep = absd <= med
    dz = np.where(keep, absd, 0.0)
    s = dz / np.maximum(dz.max(axis=-1, keepdims=True), 1e-30)
    d3 = (np.where(keep, diff, 0.0) * (1.0 - s)).sum(-1)
    cat = np.concatenate([d[:, 1, :], d3], axis=0)
    ref = np.einsum('oc,cw->ow', w2, cat) + b2[:, None]
    got = out[b, :, h, :]
    denom = np.linalg.norm(ref) + 1e-30
    return np.linalg.norm(got - ref) / denom


def kernel(x, w1, b1, w2, b2):
    x = np.asarray(x, dtype=np.float32)
    w1 = np.asarray(w1, dtype=np.float32)
    b1 = np.asarray(b1, dtype=np.float32)
    w2 = np.asarray(w2, dtype=np.float32)
    b2 = np.asarray(b2, dtype=np.float32)
    nc = _get_program()
    in_maps = _host_inputs(x, w1, b1, w2, b2)
    for attempt in range(3):
        res = run_bass_kernel_spmd(nc, in_maps, core_ids=list(range(NCORES)))
        out = np.stack([res.results[i]["out"] for i in range(NCORES)], axis=0)
        if _spot_check(out, x, w1, b1, w2, b2) < 0.05:
            break
    return out.astype(np.float32)
